# revision 5
# baseline (speedup 1.0000x reference)
"""Differentiable Preisach model on 8 Trainium2 NeuronCores.

Algorithm (beta-line aggregation + blocked clamp-scan): all relays on a
fixed-beta line of the Preisach triangle share the same reset events, so
their density-weighted sum collapses to a per-line recurrence

    z_t = min(D_t, max(z_{t-1}, U_t)),  z in [0,1] (row-normalized)

with U_t = sum_j w(beta,alpha_j) * sigmoid(1000*(h_t - alpha_j)) on rising
steps (0 on falling) and D_t = sigmoid(1000*(h_t - beta)) on falling steps
(1 on rising). 20301 hysterons -> 101 merged beta-lines, 13 lines per core.

The clamp recurrence is associative: composing f(z)=min(D,max(z,U)) stays
in the same family, so the T=2048 scan splits into B=4 independent blocks
of 512 scanned in parallel on separate SBUF partitions. Per block we need
the scan from init 0 (y) and from init 1 (Dcum); the true value with
incoming state z0 is z = max(y, min(Dcum, z0)). Both lane sets run in ONE
DVE tensor_tensor_scan over [128, 512] using a per-partition AP initial
(0 on y-lanes, 1 on Dcum-lanes); the tiny cross-block combine and the
elementwise fixup happen on the host after the output DMA.

Partition layout: block b at partitions 32b..32b+25 (PE tile_position
requires 32-aligned PSUM output tiles): +l (l<13) y-lane of line l,
+13+l Dcum-lane, +26..31 unused. The 4 matmuls (one per block, stationary
wu duplicated [w|w] -> 26 cols) write U' for both lanes directly at the
right partitions. D' is a pure function of the input h, so the host ships
it pre-blocked as the scan's data1 input.

The relay rows tu = sigmoid/step(1000*(h_t - alpha_j)) are produced by
three engines in parallel on disjoint column ranges for load balance:
ACT does SA columns exactly (sigmoid, one table), Pool and DVE do the
rest as hard steps (is_ge); at temp 1e-3 the sigmoid is a near-step and
the hard columns cost ~7e-4 extra error (measured 4.2e-3 total vs the
2e-2 gate, dominated by the 201->101 level merge).

Per core per repeat: ACT ~0.83us, Pool ~0.83us, DVE ~0.8us (scan+step),
PE 4 matmuls, one 64KB output DMA. Host folds density/mesh into the
level grid, builds gated field rows, and combines/sums the output.
"""

import numpy as np

import concourse.bass as bass
import concourse.mybir as mybir
from concourse.bass_utils import run_bass_kernel_spmd

T = 2048
NCORES = 8
L = 201              # raw beta/alpha grid levels (-1..1 step 0.01)
LB = 101             # merged beta lines (pairs)
LA = 101             # merged alpha levels (pairs)
RPC = 13             # beta lines per core (8*13 = 104 >= 101)
NB = 4               # time blocks
TB = T // NB         # 512 columns per block
BIG = 20000.0        # f16-safe saturation for gated field rows
SA = 768             # tu columns computed exactly (ACT sigmoid)
SP = 992             # tu columns as hard steps on Pool
SD = T - SA - SP     # tu columns as hard steps on DVE
F32 = mybir.dt.float32
F16 = mybir.dt.float16

_prog_cache = {}
_srows = []          # per-core 2*S_beta row scales (set by _prepare_in_maps)


def _build_program(state_bf16: bool = True, repeats: int = 1):
    nc = bass.Bass("TRN2", target_bir_lowering=False, debug=False)

    hh = nc.dram_tensor("hh", [128, T], F16, kind="ExternalInput").ap()
    wu = nc.dram_tensor("wu", [128, 2 * RPC], F16, kind="ExternalInput").ap()
    dps = nc.dram_tensor("dps", [128, TB], F16, kind="ExternalInput").ap()
    initv = nc.dram_tensor("initv", [128, 1], F32, kind="ExternalInput").ap()
    abias = nc.dram_tensor("abias", [128, 1], F32, kind="ExternalInput").ap()
    asc = nc.dram_tensor("asc", [128, 1], F32, kind="ExternalInput").ap()
    outp = nc.dram_tensor("outp", [128, TB], F16, kind="ExternalOutput").ap()

    sigmoid = mybir.ActivationFunctionType.Sigmoid
    amax = mybir.AluOpType.max
    amin = mybir.AluOpType.min
    is_ge = mybir.AluOpType.is_ge

    NIN = 6 * 16  # dma_sem target once all input DMAs land

    from contextlib import ExitStack
    with ExitStack() as ctx:
        ent = ctx.enter_context
        hh_t = ent(nc.sbuf_tensor("hh_t", [128, T], F16))
        wu_t = ent(nc.sbuf_tensor("wu_t", [128, 2 * RPC], F16))
        dps_t = ent(nc.sbuf_tensor("dps_t", [128, TB], F16))
        initv_t = ent(nc.sbuf_tensor("initv_t", [128, 1], F32))
        abias_t = ent(nc.sbuf_tensor("abias_t", [128, 1], F32))
        asc_t = ent(nc.sbuf_tensor("asc_t", [128, 1], F32))
        tu = [ent(nc.sbuf_tensor(f"tu{i}", [128, T], F16)) for i in range(2)]
        zb = [ent(nc.sbuf_tensor(f"z{i}", [128, TB], F16)) for i in range(4)]
        ps = [ent(nc.psum_tensor(f"ps{i}", [128, TB], F32)) for i in range(2)]
        dma_sem = ent(nc.semaphore("dma_sem"))
        act_sem = ent(nc.semaphore("act_sem"))
        pool_sem = ent(nc.semaphore("pool_sem"))
        pe_sem = ent(nc.semaphore("pe_sem"))
        dve_sem = ent(nc.semaphore("dve_sem"))
        odma_sem = ent(nc.semaphore("odma_sem"))
        block = ent(nc.Block())

        # per repeat: act +1, pool +1 (+2 once), pe +4, dve +2, odma +16

        @block.sync
        def _(sync):
            for t, d in [(hh_t, hh), (wu_t, wu), (dps_t, dps),
                         (initv_t, initv), (abias_t, abias), (asc_t, asc)]:
                sync.dma_start(t[:], d[:]).then_inc(dma_sem, 16)
            for r in range(repeats):
                sync.wait_ge(dve_sem, 2 * r + 2)
                sync.dma_start(outp[:], zb[r % 4][:]).then_inc(odma_sem, 16)
            sync.wait_ge(odma_sem, 16 * repeats)
            sync.wait_ge(dma_sem, NIN)

        @block.scalar
        def _(scalar):
            for r in range(repeats):
                # exact soft relay rows: sigmoid(1000*(h - alpha_j))
                a = scalar.activation(tu[r % 2][:, 0:SA], hh_t[:, 0:SA],
                                      sigmoid, bias=abias_t[:, 0:1], scale=2.0)
                if r == 0:
                    a._wait_ge(dma_sem, NIN)
                elif r >= 2:
                    # matmuls of r-2 released tu[r%2]
                    a._wait_ge(pe_sem, 4 * (r - 1))
                a.then_inc(act_sem, 1)

        @block.gpsimd
        def _(pool):
            for r in range(repeats):
                # hard-step relay rows: h >= alpha_j
                st = pool.tensor_scalar(tu[r % 2][:, SA:SA + SP],
                                        hh_t[:, SA:SA + SP],
                                        asc_t[:, 0:1], None, is_ge)
                if r == 0:
                    st._wait_ge(dma_sem, NIN)
                elif r >= 2:
                    st._wait_ge(pe_sem, 4 * (r - 1))
                st.then_inc(pool_sem, 1)

        @block.tensor
        def _(tensor):
            # both PSUM gap-memsets must land before the first matmuls
            tensor.nop()._wait_ge(pool_sem, 2)
            for r in range(repeats):
                if r >= 2:
                    # scan r-2 released ps[r%2]
                    tensor.nop()._wait_ge(dve_sem, 2 * r - 2)
                for b in range(NB):
                    # U' for block b, both lane copies (wu = [w|w]):
                    # ps[32b : 32b+26, :] = wu.T @ tu[:, 512b : 512b+512]
                    mm = tensor.matmul(
                        ps[r % 2][32 * b:32 * b + 2 * RPC, :],
                        wu_t[:],
                        tu[r % 2][:, TB * b:TB * (b + 1)],
                        start=True, stop=True, tile_position=(0, 32 * b))
                    # block column ranges: b0 in ACT range, b1 needs Pool,
                    # b3 needs DVE step; earlier waits imply the rest.
                    if b == 0:
                        mm._wait_ge(act_sem, r + 1)
                    elif b == 1:
                        mm._wait_ge(pool_sem, r + 3)
                    elif b == 3:
                        mm._wait_ge(dve_sem, 2 * r + 1)
                    mm.then_inc(pe_sem, 1)

        @block.vector
        def _(vector):
            # PSUM gap partitions (32b+26..32b+31) are never written by the
            # matmuls; zero them once so the scan reads defined values.
            # (GPSIMD cannot access PSUM, so these live on DVE.)
            vector.memset(ps[0][:], 0.0).then_inc(pool_sem, 1)
            vector.memset(ps[1][:], 0.0).then_inc(pool_sem, 1)
            for r in range(repeats):
                if r >= 4:
                    # output DMA r-4 released zb[r%4]
                    vector.nop(nofuse=True)._wait_ge(odma_sem, 16 * (r - 3))
                st = vector.tensor_scalar(tu[r % 2][:, SA + SP:T],
                                          hh_t[:, SA + SP:T],
                                          asc_t[:, 0:1], None, is_ge)
                if r == 0:
                    st._wait_ge(dma_sem, NIN)
                elif r >= 2:
                    st._wait_ge(pe_sem, 4 * (r - 1))
                st.then_inc(dve_sem, 1)
                # blocked clamp-scan, y-lanes (init 0) + Dcum-lanes (init 1)
                sc = vector.tensor_tensor_scan(
                    zb[r % 4][:], ps[r % 2][:], dps_t[:],
                    initial=initv_t[:, 0:1], op0=amax, op1=amin)
                sc._wait_ge(pe_sem, 4 * r + 4)
                sc.then_inc(dve_sem, 1)

    return nc


def _prepare_in_maps(h, density, mesh, state_bf16: bool = True):
    hf = np.asarray(h, dtype=np.float64).reshape(-1)
    prev = np.empty_like(hf)
    prev[0] = 0.0
    prev[1:] = hf[:-1]
    rising = hf > prev

    hup_row = np.where(rising, 500.0 * hf, -BIG).astype(np.float16)
    hh_rep = np.ascontiguousarray(np.broadcast_to(hup_row, (128, T)))

    # level grid: quantize mesh coords to the 0.01 grid, accumulate density
    mesh = np.asarray(mesh, dtype=np.float64)
    density = np.asarray(density, dtype=np.float64)
    lev = np.round((mesh + 1.0) / 0.01).astype(np.int64)   # [M,2] (beta, alpha)
    rho_grid = np.zeros((L, L))
    np.add.at(rho_grid, (lev[:, 0], lev[:, 1]), density)
    alpha_levels = -1.0 + 0.01 * np.arange(L)

    # merge beta-line pairs 201 -> 101, then alpha pairs 201 -> 101
    rho_gb = np.zeros((LB, L))
    rho_gb[:100] = rho_grid[0:200:2] + rho_grid[1:200:2]
    rho_gb[100] = rho_grid[200]
    beta_m = np.zeros(LB)
    beta_m[:100] = 0.5 * (alpha_levels[0:200:2] + alpha_levels[1:200:2])
    beta_m[100] = alpha_levels[200]
    rho_m = np.zeros((LB, LA))
    alpha_m = np.zeros(LA)
    rho_m[:, :100] = rho_gb[:, 0:200:2] + rho_gb[:, 1:200:2]
    alpha_m[:100] = 0.5 * (alpha_levels[0:200:2] + alpha_levels[1:200:2])
    rho_m[:, 100] = rho_gb[:, 200]
    alpha_m[100] = alpha_levels[200]

    abias = np.full((128, 1), -1.0e9, np.float32)
    abias[:LA, 0] = (-1000.0 * alpha_m).astype(np.float32)
    asc = np.full((128, 1), 60000.0, np.float32)
    asc[:LA, 0] = (500.0 * alpha_m).astype(np.float16).astype(np.float32)
    initv = np.zeros((128, 1), np.float32)
    for b in range(NB):
        initv[32 * b + RPC:32 * b + 2 * RPC, 0] = 1.0

    def _sig(x):
        return 1.0 / (1.0 + np.exp(-np.clip(x, -500.0, 500.0)))

    in_maps = []
    _srows.clear()
    for c in range(NCORES):
        rows = np.arange(c * RPC, (c + 1) * RPC)
        wu_c = np.zeros((128, 2 * RPC), np.float32)
        dps_c = np.ones((128, TB), np.float64)
        srow_c = np.zeros(RPC, np.float64)
        for p, row in enumerate(rows):
            if row < LB:
                s_row = rho_m[row].sum()
                srow_c[p] = 2.0 * s_row
                if s_row > 0:
                    wu_c[:LA, p] = rho_m[row] / s_row
                    wu_c[:LA, RPC + p] = wu_c[:LA, p]
                # D' for this line, blocked by time
                dline = np.where(rising, 1.0,
                                 _sig(1000.0 * (hf - beta_m[row])))
                for b in range(NB):
                    seg = dline[TB * b:TB * (b + 1)]
                    dps_c[32 * b + p] = seg
                    dps_c[32 * b + RPC + p] = seg
        _srows.append(srow_c)
        in_maps.append({
            "hh": hh_rep,
            "wu": wu_c.astype(np.float16),
            "dps": dps_c.astype(np.float16),
            "initv": initv,
            "abias": abias,
            "asc": asc,
        })
    return in_maps


def _postprocess(results, h, density):
    density = np.asarray(density, dtype=np.float64)
    total = np.zeros(T)
    for c in range(NCORES):
        z = np.asarray(results[c]["outp"], dtype=np.float64)   # [128, TB]
        y = np.empty((RPC, NB, TB))
        dc = np.empty((RPC, NB, TB))
        for b in range(NB):
            y[:, b] = z[32 * b:32 * b + RPC]
            dc[:, b] = z[32 * b + RPC:32 * b + 2 * RPC]
        # cross-block combine: incoming state z0 per block, then fixup
        z0 = np.zeros((RPC, NB))
        for b in range(1, NB):
            z0[:, b] = np.minimum(dc[:, b - 1, -1],
                                  np.maximum(z0[:, b - 1], y[:, b - 1, -1]))
        zfull = np.maximum(y, np.minimum(dc, z0[:, :, None])).reshape(RPC, T)
        total += (_srows[c][:, None] * zfull).sum(axis=0)
    m = total / density.sum() - 1.0
    h32 = np.asarray(h, dtype=np.float32).reshape(T, 1)
    return (m.astype(np.float32).reshape(T, 1) + h32).astype(np.float32)


def kernel(h, density, mesh, _state_bf16=True):
    key = bool(_state_bf16)
    if key not in _prog_cache:
        _prog_cache[key] = _build_program(key)
    nc = _prog_cache[key]
    in_maps = _prepare_in_maps(h, density, mesh, key)
    res = run_bass_kernel_spmd(nc, in_maps, core_ids=list(range(NCORES)))
    return _postprocess(res.results, h, density)


# revision 7
# speedup vs baseline: 8.4842x; 8.4842x over previous
"""Differentiable Preisach model on 8 Trainium2 NeuronCores.

Algorithm (beta-line aggregation + blocked clamp-scan): all relays on a
fixed-beta line of the Preisach triangle share the same reset events, so
their density-weighted sum collapses to a per-line recurrence

    z_t = min(D_t, max(z_{t-1}, U_t)),  z in [0,1] (row-normalized)

with U_t = sum_j w(beta,alpha_j) * sigmoid(1000*(h_t - alpha_j)) on rising
steps (0 on falling) and D_t = sigmoid(1000*(h_t - beta)) on falling steps
(1 on rising). 20301 hysterons -> 101 merged beta-lines, 13 lines per core.

The clamp recurrence is associative: composing f(z)=min(D,max(z,U)) stays
in the same family, so the T=2048 scan splits into B=4 independent blocks
of 512 scanned in parallel on separate SBUF partitions. Per block we need
the scan from init 0 (y) and from init 1 (Dcum); the true value with
incoming state z0 is z = max(y, min(Dcum, z0)). Both lane sets run in ONE
DVE tensor_tensor_scan over [128, 512] using a per-partition AP initial
(0 on y-lanes, 1 on Dcum-lanes); the tiny cross-block combine and the
elementwise fixup happen on the host after the output DMA.

Partition layout: block b at partitions 32b..32b+25 (PE tile_position
requires 32-aligned PSUM output tiles): +l (l<13) y-lane of line l,
+13+l Dcum-lane, +26..31 unused. The 4 matmuls (one per block, stationary
wu duplicated [w|w] -> 26 cols) write U' for both lanes directly at the
right partitions. D' is a pure function of the input h, so the host ships
it pre-blocked as the scan's data1 input.

The relay rows tu = sigmoid/step(1000*(h_t - alpha_j)) are produced by
three engines in parallel on disjoint column ranges for load balance:
ACT does SA columns exactly (sigmoid, one table), Pool and DVE do the
rest as hard steps (is_ge); at temp 1e-3 the sigmoid is a near-step and
the hard columns cost ~7e-4 extra error (measured 4.2e-3 total vs the
2e-2 gate, dominated by the 201->101 level merge).

Per core per repeat: ACT ~0.83us, Pool ~0.83us, DVE ~0.8us (scan+step),
PE 4 matmuls, one 64KB output DMA. Host folds density/mesh into the
level grid, builds gated field rows, and combines/sums the output.
"""

import numpy as np

import concourse.bass as bass
import concourse.mybir as mybir
from concourse.bass_utils import run_bass_kernel_spmd

T = 2048
NCORES = 8
L = 201              # raw beta/alpha grid levels (-1..1 step 0.01)
LB = 101             # merged beta lines (pairs)
LA = 101             # merged alpha levels (pairs)
RPC = 13             # beta lines per core (8*13 = 104 >= 101)
NB = 4               # time blocks
TB = T // NB         # 512 columns per block
BIG = 20000.0        # f16-safe saturation for gated field rows
SA = 1024            # tu columns computed exactly (ACT sigmoid)
SP = 0               # tu columns as hard steps on Pool
SD = T - SA - SP     # tu columns as hard steps on DVE
F32 = mybir.dt.float32
F16 = mybir.dt.float16

_prog_cache = {}
_srows = []          # per-core 2*S_beta row scales (set by _prepare_in_maps)


def _build_program(state_bf16: bool = True, repeats: int = 1):
    nc = bass.Bass("TRN2", target_bir_lowering=False, debug=False)

    hh = nc.dram_tensor("hh", [128, T], F16, kind="ExternalInput").ap()
    wu = nc.dram_tensor("wu", [128, 2 * RPC], F16, kind="ExternalInput").ap()
    dps = nc.dram_tensor("dps", [128, TB], F16, kind="ExternalInput").ap()
    initv = nc.dram_tensor("initv", [128, 1], F32, kind="ExternalInput").ap()
    abias = nc.dram_tensor("abias", [128, 1], F32, kind="ExternalInput").ap()
    asc = nc.dram_tensor("asc", [128, 1], F32, kind="ExternalInput").ap()
    outp = nc.dram_tensor("outp", [128, TB], F16, kind="ExternalOutput").ap()

    sigmoid = mybir.ActivationFunctionType.Sigmoid
    amax = mybir.AluOpType.max
    amin = mybir.AluOpType.min
    is_ge = mybir.AluOpType.is_ge

    NIN = 6 * 16  # dma_sem target once all input DMAs land

    from contextlib import ExitStack
    with ExitStack() as ctx:
        ent = ctx.enter_context
        hh_t = ent(nc.sbuf_tensor("hh_t", [128, T], F16))
        wu_t = ent(nc.sbuf_tensor("wu_t", [128, 2 * RPC], F16))
        dps_t = ent(nc.sbuf_tensor("dps_t", [128, TB], F16))
        initv_t = ent(nc.sbuf_tensor("initv_t", [128, 1], F32))
        abias_t = ent(nc.sbuf_tensor("abias_t", [128, 1], F32))
        asc_t = ent(nc.sbuf_tensor("asc_t", [128, 1], F32))
        tu = [ent(nc.sbuf_tensor(f"tu{i}", [128, T], F16)) for i in range(2)]
        zb = [ent(nc.sbuf_tensor(f"z{i}", [128, TB], F16)) for i in range(4)]
        ps = [ent(nc.psum_tensor(f"ps{i}", [128, TB], F32)) for i in range(2)]
        dma_sem = ent(nc.semaphore("dma_sem"))
        act_sem = ent(nc.semaphore("act_sem"))
        pool_sem = ent(nc.semaphore("pool_sem"))
        pe_sem = ent(nc.semaphore("pe_sem"))
        dve_sem = ent(nc.semaphore("dve_sem"))
        odma_sem = ent(nc.semaphore("odma_sem"))
        block = ent(nc.Block())

        # per repeat: act +1, pool +1 (if SP), pe +4, dve +DINC, odma +16
        DINC = 2 if SD > 0 else 1   # dve_sem increments per repeat

        @block.sync
        def _(sync):
            for t, d in [(hh_t, hh), (wu_t, wu), (dps_t, dps),
                         (initv_t, initv), (abias_t, abias), (asc_t, asc)]:
                sync.dma_start(t[:], d[:]).then_inc(dma_sem, 16)
            for r in range(repeats):
                sync.wait_ge(dve_sem, DINC * (r + 1))
                sync.dma_start(outp[:], zb[r % 4][:]).then_inc(odma_sem, 16)
            sync.wait_ge(odma_sem, 16 * repeats)
            sync.wait_ge(dma_sem, NIN)

        @block.scalar
        def _(scalar):
            for r in range(repeats):
                # exact soft relay rows: sigmoid(1000*(h - alpha_j))
                a = scalar.activation(tu[r % 2][:, 0:SA], hh_t[:, 0:SA],
                                      sigmoid, bias=abias_t[:, 0:1], scale=2.0)
                if r == 0:
                    a._wait_ge(dma_sem, NIN)
                elif r >= 2:
                    # matmuls of r-2 released tu[r%2]
                    a._wait_ge(pe_sem, 4 * (r - 1))
                a.then_inc(act_sem, 1)

        if SP > 0:
            @block.gpsimd
            def _(pool):
                for r in range(repeats):
                    # hard-step relay rows: h >= alpha_j
                    st = pool.tensor_scalar(tu[r % 2][:, SA:SA + SP],
                                            hh_t[:, SA:SA + SP],
                                            asc_t[:, 0:1], None, is_ge)
                    if r == 0:
                        st._wait_ge(dma_sem, NIN)
                    elif r >= 2:
                        st._wait_ge(pe_sem, 4 * (r - 1))
                    st.then_inc(pool_sem, 1)

        @block.tensor
        def _(tensor):
            # both PSUM gap-memsets must land before the first matmuls
            tensor.nop()._wait_ge(pool_sem, 2)
            for r in range(repeats):
                if r >= 2:
                    # scan r-2 released ps[r%2]
                    tensor.nop()._wait_ge(dve_sem, DINC * (r - 1))
                waited = set()
                for b in range(NB):
                    # U' for block b, both lane copies (wu = [w|w]):
                    # ps[32b : 32b+26, :] = wu.T @ tu[:, 512b : 512b+512]
                    mm = tensor.matmul(
                        ps[r % 2][32 * b:32 * b + 2 * RPC, :],
                        wu_t[:],
                        tu[r % 2][:, TB * b:TB * (b + 1)],
                        start=True, stop=True, tile_position=(0, 32 * b))
                    # wait on each producer whose column range overlaps this
                    # block, the first time it is needed (PE is in-order)
                    lo, hi = TB * b, TB * (b + 1)
                    if lo < SA and "act" not in waited:
                        waited.add("act")
                        mm._wait_ge(act_sem, r + 1)
                    elif SP > 0 and lo < SA + SP and hi > SA and "pool" not in waited:
                        waited.add("pool")
                        mm._wait_ge(pool_sem, r + 3)
                    elif SD > 0 and hi > SA + SP and "dve" not in waited:
                        waited.add("dve")
                        mm._wait_ge(dve_sem, DINC * r + 1)
                    mm.then_inc(pe_sem, 1)

        @block.vector
        def _(vector):
            # PSUM gap partitions (32b+26..32b+31) are never written by the
            # matmuls; zero them once so the scan reads defined values.
            # (GPSIMD cannot access PSUM, so these live on DVE.)
            vector.memset(ps[0][:], 0.0).then_inc(pool_sem, 1)
            vector.memset(ps[1][:], 0.0).then_inc(pool_sem, 1)
            for r in range(repeats):
                if r >= 4:
                    # output DMA r-4 released zb[r%4]
                    vector.nop(nofuse=True)._wait_ge(odma_sem, 16 * (r - 3))
                if SD > 0:
                    st = vector.tensor_scalar(tu[r % 2][:, SA + SP:T],
                                              hh_t[:, SA + SP:T],
                                              asc_t[:, 0:1], None, is_ge)
                    if r == 0:
                        st._wait_ge(dma_sem, NIN)
                    elif r >= 2:
                        st._wait_ge(pe_sem, 4 * (r - 1))
                    st.then_inc(dve_sem, 1)
                # blocked clamp-scan, y-lanes (init 0) + Dcum-lanes (init 1)
                sc = vector.tensor_tensor_scan(
                    zb[r % 4][:], ps[r % 2][:], dps_t[:],
                    initial=initv_t[:, 0:1], op0=amax, op1=amin)
                sc._wait_ge(pe_sem, 4 * r + 4)
                sc.then_inc(dve_sem, 1)

    return nc


def _prepare_in_maps(h, density, mesh, state_bf16: bool = True):
    hf = np.asarray(h, dtype=np.float64).reshape(-1)
    prev = np.empty_like(hf)
    prev[0] = 0.0
    prev[1:] = hf[:-1]
    rising = hf > prev

    hup_row = np.where(rising, 500.0 * hf, -BIG).astype(np.float16)
    hh_rep = np.ascontiguousarray(np.broadcast_to(hup_row, (128, T)))

    # level grid: quantize mesh coords to the 0.01 grid, accumulate density
    mesh = np.asarray(mesh, dtype=np.float64)
    density = np.asarray(density, dtype=np.float64)
    lev = np.round((mesh + 1.0) / 0.01).astype(np.int64)   # [M,2] (beta, alpha)
    rho_grid = np.zeros((L, L))
    np.add.at(rho_grid, (lev[:, 0], lev[:, 1]), density)
    alpha_levels = -1.0 + 0.01 * np.arange(L)

    # merge beta-line pairs 201 -> 101, then alpha pairs 201 -> 101
    rho_gb = np.zeros((LB, L))
    rho_gb[:100] = rho_grid[0:200:2] + rho_grid[1:200:2]
    rho_gb[100] = rho_grid[200]
    beta_m = np.zeros(LB)
    beta_m[:100] = 0.5 * (alpha_levels[0:200:2] + alpha_levels[1:200:2])
    beta_m[100] = alpha_levels[200]
    rho_m = np.zeros((LB, LA))
    alpha_m = np.zeros(LA)
    rho_m[:, :100] = rho_gb[:, 0:200:2] + rho_gb[:, 1:200:2]
    alpha_m[:100] = 0.5 * (alpha_levels[0:200:2] + alpha_levels[1:200:2])
    rho_m[:, 100] = rho_gb[:, 200]
    alpha_m[100] = alpha_levels[200]

    abias = np.full((128, 1), -1.0e9, np.float32)
    abias[:LA, 0] = (-1000.0 * alpha_m).astype(np.float32)
    asc = np.full((128, 1), 60000.0, np.float32)
    asc[:LA, 0] = (500.0 * alpha_m).astype(np.float16).astype(np.float32)
    initv = np.zeros((128, 1), np.float32)
    for b in range(NB):
        initv[32 * b + RPC:32 * b + 2 * RPC, 0] = 1.0

    def _sig(x):
        return 1.0 / (1.0 + np.exp(-np.clip(x, -500.0, 500.0)))

    in_maps = []
    _srows.clear()
    for c in range(NCORES):
        rows = np.arange(c * RPC, (c + 1) * RPC)
        wu_c = np.zeros((128, 2 * RPC), np.float32)
        dps_c = np.ones((128, TB), np.float64)
        srow_c = np.zeros(RPC, np.float64)
        for p, row in enumerate(rows):
            if row < LB:
                s_row = rho_m[row].sum()
                srow_c[p] = 2.0 * s_row
                if s_row > 0:
                    wu_c[:LA, p] = rho_m[row] / s_row
                    wu_c[:LA, RPC + p] = wu_c[:LA, p]
                # D' for this line, blocked by time
                dline = np.where(rising, 1.0,
                                 _sig(1000.0 * (hf - beta_m[row])))
                for b in range(NB):
                    seg = dline[TB * b:TB * (b + 1)]
                    dps_c[32 * b + p] = seg
                    dps_c[32 * b + RPC + p] = seg
        _srows.append(srow_c)
        in_maps.append({
            "hh": hh_rep,
            "wu": wu_c.astype(np.float16),
            "dps": dps_c.astype(np.float16),
            "initv": initv,
            "abias": abias,
            "asc": asc,
        })
    return in_maps


def _postprocess(results, h, density):
    density = np.asarray(density, dtype=np.float64)
    total = np.zeros(T)
    for c in range(NCORES):
        z = np.asarray(results[c]["outp"], dtype=np.float64)   # [128, TB]
        y = np.empty((RPC, NB, TB))
        dc = np.empty((RPC, NB, TB))
        for b in range(NB):
            y[:, b] = z[32 * b:32 * b + RPC]
            dc[:, b] = z[32 * b + RPC:32 * b + 2 * RPC]
        # cross-block combine: incoming state z0 per block, then fixup
        z0 = np.zeros((RPC, NB))
        for b in range(1, NB):
            z0[:, b] = np.minimum(dc[:, b - 1, -1],
                                  np.maximum(z0[:, b - 1], y[:, b - 1, -1]))
        zfull = np.maximum(y, np.minimum(dc, z0[:, :, None])).reshape(RPC, T)
        total += (_srows[c][:, None] * zfull).sum(axis=0)
    m = total / density.sum() - 1.0
    h32 = np.asarray(h, dtype=np.float32).reshape(T, 1)
    return (m.astype(np.float32).reshape(T, 1) + h32).astype(np.float32)


def kernel(h, density, mesh, _state_bf16=True):
    key = bool(_state_bf16)
    if key not in _prog_cache:
        _prog_cache[key] = _build_program(key)
    nc = _prog_cache[key]
    in_maps = _prepare_in_maps(h, density, mesh, key)
    res = run_bass_kernel_spmd(nc, in_maps, core_ids=list(range(NCORES)))
    return _postprocess(res.results, h, density)
